# revision 13
# baseline (speedup 1.0000x reference)
"""BlockDiagonalLowRankLinear Trainium2 kernel.

y = BlockDiag(blocks) @ x + U @ (V.T @ x), scaled by alpha, plus bias.

Shapes (full problem):
  x      [4, 2048, 4096] f32   -> flattened to [8192, 4096]
  blocks [16, 256, 256]  f32   (per-block [out, in])
  U      [4096, 64] f32, V [4096, 64] f32, bias [4096] f32, alpha [1] f32
  out    [4, 2048, 4096] f32
Sharding: data-parallel over tokens (1024/core, replicated params, no
collectives) — minimizes HBM traffic (16MB in + 16MB out per core).

Per-core algorithm (T=1024 tokens, D=4096, R=64, NB=16, bi=bo=256):
  - Setup: stage params, PE-transpose blocks -> blocks^T and U -> U^T
    (f32r transpose mode, 1.5 cyc/row), alpha-scale them into bf16
    stationaries.  bias is NOT broadcast: it becomes row 64 of a
    [65, D] "u65" operand so the U-term matmul adds it for free
    (lhsT gets a matching all-ones row 65).
  - Steady state, 4 slabs of 256 tokens, software-pipelined:
    Phase A(s): PE-transpose x (f32r, 1.5 cyc/row) -> Act copies convert
      PSUM -> bf16 x^T; accumulate t_lr = V^T x (bf16 matmuls).
    Phase B(s): per (t-chunk, o-chunk) PSUM tile: 4 block-diagonal bf16
      matmuls (K=128) + one [65 x 512] U/bias matmul (K=65, includes the
      +bias via the ones row); DVE/Pool copy final PSUM into a [128, 4096]
      SBUF staging tile; ONE 2MB DMA per t-chunk writes out.
    B(s) is interleaved with A(s+1) per o-chunk so the PE never drains.
  - DMA: one 2MB dma_start per x t-chunk load and per out t-chunk store
    (16KB/partition contiguous lines, ~4 big DMAs per slab) to stay near
    the ~360-430 GB/s HBM-per-core roofline and minimize the 565ns/DMA
    sync-sequencer cost.

All compute matmuls are bf16 (1 cyc/row, FWL weight loads); accumulation
stays fp32 in PSUM.  bf16 quantization keeps rel-err ~1e-3 (gate: 2e-2).
"""

import numpy as np

import concourse.bacc as bacc
import concourse.bass as bass
import concourse.mybir as mybir
import concourse.tile as tile
from concourse.bass_utils import run_bass_kernel_spmd
from concourse.masks import make_identity

F32 = mybir.dt.float32
F32R = mybir.dt.float32r
BF16 = mybir.dt.bfloat16

N_CORES = 8
# --- experiment knobs (read at build time) ---
XLOAD_SPLIT = 1       # dma_starts per [128, 4096] x tile (1 -> one 2MB DMA)
STORE_ENGINE = "act"  # 'act' or 'sp': HWDGE queue for out stores
STORE_IN_FIN = False  # True: per-oc [128,512] stores right after each copy
D = 4096          # in = out features
R = 64            # low rank
NB = 16           # diagonal blocks
BI = 256          # block in/out size
NK = D // 128     # 32 i-chunks
T_CORE = 1024     # tokens per core
T_SLAB = 256      # tokens per slab
OC = 512          # output column chunk


def build(t_core: int = T_CORE, repeats: int = 1):
    nc = bacc.Bacc("TRN2", target_bir_lowering=False, debug=False)
    # f32r is bit-identical fp32; declaring inputs as f32r makes PE
    # transpose-mode run at 1.5 cyc/row instead of 2.0.
    x = nc.declare_dram_parameter("x", [t_core, D], F32R, isOutput=False)
    blocks = nc.declare_dram_parameter("blocks", [NB, BI, BI], F32R, isOutput=False)
    U = nc.declare_dram_parameter("U", [D, R], F32R, isOutput=False)
    V = nc.declare_dram_parameter("V", [D, R], F32, isOutput=False)
    bias = nc.declare_dram_parameter("bias", [D], F32, isOutput=False)
    alpha = nc.declare_dram_parameter("alpha", [1], F32, isOutput=False)
    out = nc.declare_dram_parameter("out", [t_core, D], F32, isOutput=True)

    n_slab = t_core // T_SLAB
    n_tc = T_SLAB // 128          # t-chunks per slab
    n_oc = D // OC                # 8 output chunks

    with tile.TileContext(nc) as tc:
        with (
            tc.tile_pool(name="const", bufs=1) as cpool,
            tc.tile_pool(name="psum", bufs=4, space="PSUM") as psum,
            tc.tile_pool(name="tpsum", bufs=3, space="PSUM") as tpsum,
            tc.tile_pool(name="lrpsum", bufs=1, space="PSUM") as lrpsum,
        ):
            xpool_cm = tc.tile_pool(name="xpool", bufs=2)
            xpool = xpool_cm.__enter__()
            xTpool_cm = tc.tile_pool(name="xT", bufs=2)
            xTpool = xTpool_cm.__enter__()
            opool_cm = tc.tile_pool(name="opool", bufs=2)
            opool = opool_cm.__enter__()
            spool_cm = tc.tile_pool(name="stage", bufs=1)
            spool = spool_cm.__enter__()

            ident_f32 = spool.tile([128, 128], F32)
            make_identity(nc, ident_f32[:])
            ident = cpool.tile([128, 128], F32R)
            nc.vector.tensor_copy(ident[:], ident_f32[:])

            # ---- DMAs first: x for slab 0, then params ----
            def load_xnat(s):
                t0 = s * T_SLAB
                w = D // XLOAD_SPLIT
                tiles = []
                for tcI in range(n_tc):
                    xt = xpool.tile([128, D], F32R, tag="xnat")
                    for q in range(XLOAD_SPLIT):
                        nc.sync.dma_start(
                            xt[:, q * w:(q + 1) * w],
                            x[t0 + tcI * 128: t0 + (tcI + 1) * 128,
                              q * w:(q + 1) * w])
                    tiles.append(xt)
                return tiles

            ones_t = spool.tile([1, 128], F32)
            nc.vector.memset(ones_t[:], 1.0)

            alpha_row = spool.tile([1, 1], F32)
            nc.sync.dma_start(alpha_row[:], alpha[None, :])

            xnat = load_xnat(0)

            v_stage = spool.tile([128, NK, R], F32, tag="uv")
            nc.sync.dma_start(v_stage[:], V.rearrange("(a p) r -> p a r", p=128))
            v_sb = cpool.tile([128, NK, R], BF16)
            nc.vector.tensor_copy(v_sb[:], v_stage[:])
            blk_view = blocks.rearrange("b (g p) i -> p (b g) i", p=128)

            # ---- steady-state phases (emitted interleaved below) ----
            xT_tiles = [None] * n_slab
            tlr_tiles = [None] * n_slab
            tlr_sb_tiles = [None] * n_slab

            def phaseA_transposes(s, oc, xnat_s):
                xT = xT_tiles[s]
                for pair in range(2):
                    ki0 = 4 * oc + 2 * pair
                    pt = tpsum.tile([128, 512], F32R, tag="tp")
                    for kk in range(2):
                        for tcI in range(n_tc):
                            nc.tensor.transpose(
                                pt[:, kk * 256 + tcI * 128: kk * 256 + (tcI + 1) * 128],
                                xnat_s[tcI][:, (ki0 + kk) * 128:(ki0 + kk + 1) * 128],
                                ident[:],
                            )
                    nc.scalar.copy(xT[:, ki0:ki0 + 2, :], pt[:])

            def phaseA_st1(s, oc):
                xT = xT_tiles[s]
                tlr = tlr_tiles[s]
                for kk in range(4):
                    ki = 4 * oc + kk
                    nc.tensor.matmul(
                        tlr[:], v_sb[:, ki, :], xT[:, ki, :],
                        start=(ki == 0), stop=(ki == NK - 1),
                        skip_group_check=True,
                    )

            def phaseA_group(s, oc, xnat_s):
                phaseA_transposes(s, oc, xnat_s)
                phaseA_st1(s, oc)

            def phaseA_start(s):
                xT_t = xTpool.tile([128, NK, T_SLAB], BF16, tag="xT")
                xT_tiles[s] = xT_t
                tlr_t = lrpsum.tile([R, T_SLAB], F32, tag="tlr")
                tlr_tiles[s] = tlr_t

            def phaseA_finish(s):
                # [65, T_SLAB] bf16: rows 0-63 = t_lr, row 64 = ones (bias row)
                tlr_sb = xTpool.tile([R + 1, T_SLAB], BF16, tag="tlr_sb")
                nc.vector.tensor_copy(tlr_sb[0:R, :], tlr_tiles[s][:])
                nc.vector.memset(tlr_sb[R:R + 1, :], 1.0)
                tlr_sb_tiles[s] = tlr_sb

            def phaseB_bd(s, oc, blocksT):
                """block-diagonal matmuls for o-chunk oc of slab s"""
                xT = xT_tiles[s]
                accs = []
                for tcI in range(n_tc):
                    acc = psum.tile([128, OC], F32, tag="acc")
                    for kk in range(4):
                        ki = 4 * oc + kk
                        nc.tensor.matmul(
                            acc[:, (kk // 2) * 256:(kk // 2) * 256 + 256],
                            xT[:, ki, tcI * 128:(tcI + 1) * 128],
                            blocksT[:, ki, :],
                            start=(kk == 0), stop=False,
                            skip_group_check=True,
                        )
                    accs.append(acc)
                return accs

            store_eng = nc.scalar if STORE_ENGINE == "act" else nc.sync

            def phaseB_fin(s, oc, accs, u65, osb_s):
                """U-term + bias matmul, then PSUM -> out staging copy."""
                t0 = (s % n_slab) * T_SLAB
                tlr_sb = tlr_sb_tiles[s]
                for tcI in range(n_tc):
                    acc = accs[tcI]
                    nc.tensor.matmul(
                        acc[:], tlr_sb[:, tcI * 128:(tcI + 1) * 128],
                        u65[:, 4 * oc:4 * oc + 4, :],
                        start=False, stop=True, skip_group_check=True,
                    )
                    # PSUM -> SBUF staging on DVE (Act has the x^T copies;
                    # Pool cannot access PSUM on TRN2)
                    dst = osb_s[tcI][:, oc * OC:(oc + 1) * OC]
                    nc.vector.tensor_copy(dst, acc[:])
                    if STORE_IN_FIN:
                        store_eng.dma_start(
                            out[t0 + tcI * 128: t0 + (tcI + 1) * 128,
                                oc * OC:(oc + 1) * OC],
                            dst,
                        )

            def phaseB_store(s, osb_s):
                # issue from the Act HWDGE queue: stores (which wait on the
                # PSUM copies) then never head-of-line-block the SP queue's
                # x prefetch loads
                if STORE_IN_FIN:
                    return
                t0 = (s % n_slab) * T_SLAB
                for tcI in range(n_tc):
                    store_eng.dma_start(
                        out[t0 + tcI * 128: t0 + (tcI + 1) * 128, :],
                        osb_s[tcI][:],
                    )

            # ---- slab 0 Phase A (PE busy while params finish staging) ----
            phaseA_start(0)
            for oc in range(n_oc):
                phaseA_transposes(0, oc, xnat)
            for oc in range(n_oc):
                phaseA_st1(0, oc)
            phaseA_finish(0)

            # ---- param setup on PE (transpose-mode) + alpha-scaled copies ----
            # alpha broadcast to [128, 1]
            alpha_col = cpool.tile([128, 1], F32)
            a_ps = tpsum.tile([128, 512], F32, tag="tp")
            nc.tensor.matmul(a_ps[:, :1], ones_t[:], alpha_row[:],
                             start=True, stop=True)
            nc.vector.tensor_copy(alpha_col[:], a_ps[:, :1])

            blocksT = cpool.tile([128, NK, BI], BF16)
            u65 = cpool.tile([R + 1, NK, 128], BF16)

            def setup_blocks_round(rnd):
                blk_stage = spool.tile([128, NB, BI], F32R, tag="blk")
                nc.sync.dma_start(blk_stage[:],
                                  blk_view[:, rnd * NB:(rnd + 1) * NB, :])
                for bb_ in range(NB // 2):
                    b = rnd * (NB // 2) + bb_
                    for ihalf in range(2):
                        ki = 2 * b + ihalf
                        pt = tpsum.tile([128, 512], F32R, tag="tp")
                        for g in range(2):
                            nc.tensor.transpose(
                                pt[:, g * 128:(g + 1) * 128],
                                blk_stage[:, 2 * bb_ + g, ihalf * 128:(ihalf + 1) * 128],
                                ident[:],
                            )
                        nc.vector.tensor_scalar_mul(blocksT[:, ki, :], pt[:, :256],
                                                    alpha_col[:, 0:1])

            setup_blocks_round(0)

            # bias -> row 64 of u65 (not alpha-scaled; y = alpha*(...) + bias)
            bias_stage = spool.tile([1, NK, 128], F32)
            nc.sync.dma_start(
                bias_stage[:], bias.rearrange("(o a p) -> o a p", o=1, p=128))
            nc.vector.tensor_copy(u65[R:R + 1, :, :], bias_stage[:])

            u_stage = spool.tile([128, NK, R], F32R, tag="uv")
            nc.sync.dma_start(u_stage[:], U.rearrange("(a p) r -> p a r", p=128))
            for j in range(NK // 4):
                up = tpsum.tile([128, 512], F32R, tag="tp")
                for q in range(4):
                    a = 4 * j + q
                    nc.tensor.transpose(
                        up[:R, q * 128:(q + 1) * 128], u_stage[:, a, :], ident[:])
                nc.vector.tensor_scalar_mul(u65[0:R, 4 * j:4 * j + 4, :], up[:R, :],
                                            alpha_col[:R, 0:1])

            setup_blocks_round(1)

            # ---- software-pipelined steady loop ----
            total = repeats * n_slab
            for it in range(total):
                s = it % n_slab
                nxt = it + 1
                if nxt < total:
                    xnat = load_xnat(nxt % n_slab)
                    phaseA_start(nxt % n_slab)
                osb = []
                for tcI in range(n_tc):
                    osb_t = opool.tile([128, D], F32, tag="osb")
                    osb.append(osb_t)
                pending = None
                for oc in range(n_oc):
                    if nxt < total:
                        phaseA_group(nxt % n_slab, oc, xnat)
                    accs = phaseB_bd(s, oc, blocksT)
                    if pending is not None:
                        phaseB_fin(s, pending[0], pending[1], u65, osb)
                    pending = (oc, accs)
                phaseB_fin(s, pending[0], pending[1], u65, osb)
                phaseB_store(s, osb)
                if nxt < total:
                    phaseA_finish(nxt % n_slab)

            spool_cm.__exit__(None, None, None)
            opool_cm.__exit__(None, None, None)
            xTpool_cm.__exit__(None, None, None)
            xpool_cm.__exit__(None, None, None)
    nc.compile()
    return nc


def check_waits(nc, verbose=True):
    bad = 0
    for fn in nc.m.functions:
        for bb in fn.blocks:
            for ins in bb.instructions:
                tname = type(ins).__name__
                if tname == "InstDrain":
                    continue
                nw = len(ins.sync_info.on_wait) if ins.sync_info else 0
                if tname == "InstEventSemaphore" and nw <= 2:
                    continue
                if nw > 1:
                    bad += 1
                    if verbose:
                        print("MULTI-WAIT", tname, ins.name,
                              [(w.ant_name, w.wait_value) for w in ins.sync_info.on_wait])
    return bad


_NC_CACHE = {}


def _get_nc(t_core, repeats=1):
    key = (t_core, repeats)
    if key not in _NC_CACHE:
        _NC_CACHE[key] = build(t_core, repeats)
    return _NC_CACHE[key]


def kernel(x, blocks, U, V, bias, alpha):
    batch_dims = x.shape[:-1]
    x_flat = np.ascontiguousarray(x.reshape(-1, D).astype(np.float32))
    n_tok = x_flat.shape[0]
    t_core = n_tok // N_CORES
    nc = _get_nc(t_core)

    blocks = np.ascontiguousarray(blocks, dtype=np.float32)
    U = np.ascontiguousarray(U, dtype=np.float32)
    V = np.ascontiguousarray(V, dtype=np.float32)
    bias = np.ascontiguousarray(bias, dtype=np.float32)
    alpha = np.ascontiguousarray(alpha, dtype=np.float32)

    in_maps = [
        {
            "x": x_flat[c * t_core:(c + 1) * t_core],
            "blocks": blocks, "U": U, "V": V, "bias": bias, "alpha": alpha,
        }
        for c in range(N_CORES)
    ]
    res = run_bass_kernel_spmd(nc, in_maps, list(range(N_CORES)))
    out = np.concatenate([res.results[c]["out"] for c in range(N_CORES)], axis=0)
    return out.reshape(*batch_dims, D)
